# revision 1
# baseline (speedup 1.0000x reference)
"""3-layer GCN + mean-pool + classifier for Trainium2, SPMD on 8 NeuronCores.

Self-contained: kernel(**inputs) takes the full-size numpy inputs, does the
host-side graph partitioning, builds/compiles a Bass/Tile kernel, runs it on
cores 0-7 via run_bass_kernel_spmd, and returns the [128, 3] log-softmax
output.

Distribution: nodes are dst-sharded across the 8 cores. Per GCN layer each
core computes t' = dinv * (h @ W) for its shard (TensorE), the shards are
AllGathered into a full node-major table, each core dma_gathers its in-edges'
source rows (256B rows) and dma_scatter_adds them into its local accumulator
table. Scatter calls are organized into rounds of unique destination rows
(the SDMA CCE read-modify-write loses updates when one call carries duplicate
indices); nodes with degree > LV overflow into chained virtual rows that are
merged back hierarchically. HW limits found empirically: gather index values
must be < 8192 (16 source buckets), gather calls <= 1024 indices, scatter
calls <= 2048. The GCN normalization deg^-1/2 (A+I) deg^-1/2 factorizes into
a pre-scale of t' and a post-scale of the aggregate, so no per-edge weight is
needed; self-loops become a prefill of the accumulator with t'. Mean-pooling
runs as a one-hot matmul on TensorE with an AllReduce of per-core partials;
the classifier + log_softmax run replicated on every core.
"""
import sys

sys.path.insert(0, "/opt/trn_rl_repo")

import numpy as np
import concourse.bacc as bacc
import concourse.mybir as mybir
import concourse.tile as tile
from concourse.masks import make_identity
import concourse.tile as _tile
import concourse.mybir as _mybir
from concourse.vector_clock import ScopedClock as _ScopedClock

# ---------------------------------------------------------------------------
# Workarounds: this walrus build rejects >1 sync-wait per instruction.

import concourse.tile as _tile
import concourse.mybir as _mybir
from concourse.vector_clock import ScopedClock as _ScopedClock


def _split_waits_tail(nc, inst):
    si = inst.ins.sync_info
    if si is None or not si.on_wait or len(si.on_wait) <= 1:
        return
    waits = list(si.on_wait)
    inst.ins.sync_info = _mybir.SyncInfo(on_wait=[], on_update=list(si.on_update or []))
    for w in waits:
        nop = nc.sync.nop()
        nop.ins.sync_info = _mybir.SyncInfo(on_wait=[w], on_update=[])


def _drain_and_barrier(self, tick_clock, wait_clock):
    nc = self.nc
    probe = nc.sync.nop()
    wait_clock.add_sem_waits(probe.ins, _ScopedClock({None: tick_clock.global_clock}))
    _split_waits_tail(nc, probe)
    nc.sync.drain()
    nc.all_engine_barrier()
    assert self.sems is not None
    popped = nc._tile_sem_poison_stack.pop()
    assert popped is self._sem_poison
    nc.clear_and_free_semaphores(list(self.sems.allocated().values()))
    nc.all_engine_barrier()


_tile.TileContext._drain_and_barrier = _drain_and_barrier


def fix_multiwait(nc):
    """Rewrite every >1-wait instruction into wait-nops + 1-wait instruction."""
    for f in nc.m.functions:
        for blk in f.blocks:
            insts = blk.instructions            # live list (rust-backed)
            i = 0
            while i < len(insts):
                inst = insts[i]
                si = inst.sync_info
                if si is not None and si.on_wait and len(si.on_wait) > 1:
                    waits = list(si.on_wait)
                    eng = inst.engine
                    inst.sync_info = _mybir.SyncInfo(
                        on_wait=[waits[-1]], on_update=list(si.on_update or [])
                    )
                    for j, w in enumerate(waits[:-1]):
                        nop = nc.engines[eng].nop(hint="mwfix")
                        popped = False
                        for f2 in nc.m.functions:
                            for b2 in f2.blocks:
                                l2 = b2.instructions
                                if l2 and l2[-1].name == nop.ins.name:
                                    l2.pop()
                                    popped = True
                                    break
                            if popped:
                                break
                        assert popped, "could not relocate mwfix nop"
                        nop.ins.sync_info = _mybir.SyncInfo(on_wait=[w], on_update=[])
                        insts.insert(i + j, nop.ins)
                    i += len(waits) - 1
                i += 1


# ---------------------------------------------------------------------------

import numpy as np
import concourse.bacc as bacc
import concourse.mybir as mybir
import concourse.tile as tile
from concourse.masks import make_identity

F32 = mybir.dt.float32
I16 = mybir.dt.int16
AF = mybir.ActivationFunctionType
ALU = mybir.AluOpType


def cdiv(a, b):
    return (a + b - 1) // b


def rup(a, b):
    return cdiv(a, b) * b


class Cfg:
    def __init__(self, N, E, IN, HID, G, OUT, LV=20):
        self.C = 8
        self.N, self.E, self.IN, self.HID, self.G, self.OUT = N, E, IN, HID, G, OUT
        assert N % self.C == 0
        self.NSH = N // self.C
        self.TROW = rup(self.NSH, 128)
        self.NCHK = self.TROW // 128
        self.NBUK = 16
        assert (self.C * self.TROW) % self.NBUK == 0
        self.SRCW = self.C * self.TROW // self.NBUK
        assert self.SRCW <= 8191  # HW: gather idx value must fit 13 bits
        self.LV = LV                      # rounds per row (round cap)
        assert G <= 128


def _ranks(dst):
    """rank of each element within its dst group (stable)."""
    n = len(dst)
    order = np.lexsort((np.arange(n), dst))
    sd = dst[order]
    first = np.r_[0, np.flatnonzero(np.diff(sd)) + 1]
    sizes = np.diff(np.r_[first, n])
    grp_start = np.repeat(first, sizes)
    rank_sorted = np.arange(n) - grp_start
    rank = np.empty(n, np.int64)
    rank[order] = rank_sorted
    return rank


def _wrap_cols(a):
    """[n sl ots] (n % 16 == 0) -> wrapped [128, n // 16] int16."""
    w = a.reshape(-1, 16).T  # [16, n//16]
    return np.tile(w, (8, 1)).astype(np.int16)


def prep(inputs, cfg):
    c = cfg
    x = np.asarray(inputs["x"], np.float32)
    ei = np.asarray(inputs["edge_index"], np.int64)
    batch = np.asarray(inputs["batch"], np.int64)
    W1 = np.asarray(inputs["W1"], np.float32); b1 = np.asarray(inputs["b1"], np.float32)
    W2 = np.asarray(inputs["W2"], np.float32); b2 = np.asarray(inputs["b2"], np.float32)
    W3 = np.asarray(inputs["W3"], np.float32); b3 = np.asarray(inputs["b3"], np.float32)
    Wc = np.asarray(inputs["Wc"], np.float32); bc = np.asarray(inputs["bc"], np.float32)

    src = ei[0].astype(np.int64)
    dst = ei[1].astype(np.int64)
    deg = np.bincount(dst, minlength=c.N).astype(np.float32) + 1.0
    dinv = 1.0 / np.sqrt(deg)

    HID = c.HID
    W3p = np.zeros((HID, HID), np.float32); W3p[:, : W3.shape[1]] = W3
    b3p = np.zeros((HID,), np.float32); b3p[: b3.shape[0]] = b3
    Wcp = np.zeros((HID, c.OUT), np.float32); Wcp[: Wc.shape[0]] = Wc

    core_of = src // c.NSH
    trow_src = core_of * c.TROW + (src - core_of * c.NSH)
    buk_all = trow_src // c.SRCW
    gidx_all = trow_src - buk_all * c.SRCW
    dcore = dst // c.NSH

    LV = c.LV
    DUMP = c.TROW            # dump row (pads), rows TROW..TROW+127 unused
    VBASE = c.TROW + 128     # virtual rows start here

    percore = []
    maxdeg = 0
    for ci in range(c.C):
        m = dcore == ci
        e_g = gidx_all[m]
        e_b = buk_all[m]
        d_loc = dst[m] - ci * c.NSH
        rank = _ranks(d_loc)
        maxdeg = max(maxdeg, int(rank.max(initial=0)) + 1)
        percore.append(dict(e_g=e_g, e_b=e_b, d_loc=d_loc, rank=rank,
                            lvl=rank // LV))
    NLVL = cdiv(maxdeg, LV) - 1          # number of virtual levels (>=0)
    nvr_max = [0] * NLVL                 # cross-core max vrows per level
    for pc in percore:
        for L in range(1, NLVL + 1):
            nvr_max[L - 1] = max(nvr_max[L - 1],
                                 len(np.unique(pc["d_loc"][pc["lvl"] >= L])))
    NV = [rup(max(n, 1), 128) for n in nvr_max]
    VLBASE = []
    base = VBASE
    for L in range(NLVL):
        VLBASE.append(base)
        base += NV[L]
    AGGROWS = base

    # per-core final rows + per-(core,round,bucket) counts
    R = min(LV, maxdeg)
    cnts = np.zeros((c.C, R, c.NBUK), np.int64)
    for ci in range(c.C):
        pc = percore[ci]
        vmaps = []
        for L in range(1, NLVL + 1):
            uds = np.unique(pc["d_loc"][pc["lvl"] >= L])
            vmaps.append({d: VLBASE[L - 1] + i for i, d in enumerate(uds)})
        frow = pc["d_loc"].copy()
        l = pc["lvl"]
        for L in range(1, NLVL + 1):
            if (l == L).any():
                frow[l == L] = np.array(
                    [vmaps[L - 1][d] for d in pc["d_loc"][l == L]], np.int64)
        pc["frow"] = frow
        pc["frank"] = pc["rank"] % LV
        pc["vmaps"] = vmaps
        for r in range(R):
            sel = pc["frank"] == r
            for b in range(c.NBUK):
                cnts[ci, r, b] = int((sel & (pc["e_b"] == b)).sum())
    SEG = np.zeros((R, c.NBUK), np.int64)
    for r in range(R):
        for b in range(c.NBUK):
            mx = int(cnts[:, r, b].max())
            SEG[r, b] = rup(mx, 128) if mx > 0 else 0
    # pack each round's bucket segments into bins of <= MAXMSG slots; one
    # scatter call per bin (unique rows within a round => within a bin).
    # HW limit: gather calls take at most MAXG indices, so split big segments.
    MAXMSG = 2048
    MAXG = 1024
    CALLS = []   # (bin_slots, scol, [(bucket, seg, col), ...])
    col = 0
    for r in range(R):
        cur = []
        cur_sz = 0
        scol = col
        for b in range(c.NBUK):
            seg = int(SEG[r, b])
            if seg == 0:
                continue
            while seg > 0:
                sub = min(seg, MAXG)
                if cur_sz + sub > MAXMSG:
                    CALLS.append((cur_sz, scol, cur))
                    cur, cur_sz, scol = [], 0, col
                cur.append((b, sub, col))
                cur_sz += sub
                col += sub
                seg -= sub
        if cur:
            CALLS.append((cur_sz, scol, cur))
    TOT = col
    RSZ = SEG.sum(axis=1)

    # merge calls, deepest level first: L -> L-1 -> ... -> real rows
    VM = [NV[L] for L in reversed(range(NLVL))]
    MTOT = sum(VM)
    # split each level's merge into <= MAXG sub-calls (disjoint rows => safe)
    MCALLS = []
    for v in VM:
        while v > 0:
            sub = min(v, MAXG)
            MCALLS.append(sub)
            v -= sub

    GCOLS = (TOT + MTOT) // 16
    SCOLS = (TOT + MTOT) // 16

    cnt = np.bincount(batch, minlength=c.G).astype(np.float32)
    cntinv = (1.0 / np.maximum(cnt, 1.0)).astype(np.float32)

    in_maps = []
    for ci in range(c.C):
        pc = percore[ci]
        lo, hi = ci * c.NSH, (ci + 1) * c.NSH
        xT = np.zeros((c.IN, c.TROW), np.float32)
        xT[:, : c.NSH] = x[lo:hi].T
        dv = np.zeros((c.TROW,), np.float32)
        dv[: c.NSH] = dinv[lo:hi]
        dinv2d = dv.reshape(c.NCHK, 128).T.copy()

        g_slots = np.zeros(TOT + MTOT, np.int64)           # gather idx per slot
        s_slots = np.full(TOT + MTOT, DUMP, np.int64)      # scatter idx per slot
        off = 0
        for r in range(R):
            selr = pc["frank"] == r
            for b in range(c.NBUK):
                sel = selr & (pc["e_b"] == b)
                k = int(sel.sum())
                g_slots[off: off + k] = pc["e_g"][sel]
                s_slots[off: off + k] = pc["frow"][sel]
                # sanity: unique dst within the round call
                off += int(SEG[r, b])
        assert off == TOT
        # merge slots: gather from aggb vrows, scatter to parents (deepest 1st)
        vmaps = pc["vmaps"]
        for mi, L in enumerate(reversed(range(1, NLVL + 1))):
            items = sorted(vmaps[L - 1].items(), key=lambda kv: kv[1])
            for i, (d, vr) in enumerate(items):
                g_slots[off + i] = vr
                s_slots[off + i] = vmaps[L - 2][d] if L >= 2 else d
            g_slots[off + len(items): off + VM[mi]] = DUMP
            off += VM[mi]
        assert off == TOT + MTOT

        # uniqueness check per scatter call (excluding DUMP pads)
        for bin_sz, scol, _ in CALLS:
            ss = s_slots[scol: scol + bin_sz]
            real = ss[ss != DUMP]
            assert len(np.unique(real)) == len(real), "dup within scatter call"

        gidx_w = _wrap_cols(g_slots)                       # [128, GCOLS]
        sidx_w = _wrap_cols(s_slots)

        oneh = np.zeros((c.TROW, 128), np.float32)
        oneh[np.arange(c.NSH), batch[lo:hi].astype(np.int64)] = 1.0

        bcols = np.stack([b1, b2, b3p], axis=1)
        b3rep = np.tile(b3p[None, :], (128, 1))
        bcrep = np.tile(bc[None, :], (128, 1))
        cinv = np.zeros((128, 1), np.float32)
        cinv[: c.G, 0] = cntinv

        in_maps.append(dict(
            xT=xT, dinv2d=dinv2d, gidx=gidx_w, sidx=sidx_w, oneh=oneh,
            W1d=W1, W2d=W2, W3d=W3p, bcols=bcols, b3rep=b3rep,
            Wcp=Wcp, bcrep=bcrep, cinv=cinv,
        ))

    meta = dict(R=R, CALLS=CALLS, TOT=TOT, MAXMSG=MAXMSG,
                VM=MCALLS, GCOLS=GCOLS, AGGROWS=AGGROWS, DUMP=DUMP)
    return in_maps, meta


def build(cfg, meta):
    c = cfg
    HID, G, OUT = c.HID, c.G, c.OUT
    CALLS, TOT, VM = meta["CALLS"], meta["TOT"], meta["VM"]
    GCOLS = meta["GCOLS"]
    AGGROWS = meta["AGGROWS"]
    MAXMSG = meta["MAXMSG"]
    assert max(VM) <= MAXMSG

    nc = bacc.Bacc("TRN2", num_devices=c.C, dynamic_dma_scratch_size=65536)

    def ein(name, shape, dt=F32):
        return nc.dram_tensor(name, shape, dt, kind="ExternalInput")

    xT_d = ein("xT", [c.IN, c.TROW])
    dinv_d = ein("dinv2d", [128, c.NCHK])
    gidx_d = ein("gidx", [128, GCOLS], I16)
    sidx_d = ein("sidx", [128, GCOLS], I16)
    oneh_d = ein("oneh", [c.TROW, 128])
    W1_d = ein("W1d", [c.IN, HID])
    W2_d = ein("W2d", [HID, HID])
    W3_d = ein("W3d", [HID, HID])
    bcols_d = ein("bcols", [HID, 3])
    b3rep_d = ein("b3rep", [128, HID])
    Wc_d = ein("Wcp", [HID, OUT])
    bcrep_d = ein("bcrep", [128, OUT])
    cinv_d = ein("cinv", [128, 1])

    agin_d = nc.dram_tensor("agin", [c.TROW, HID], F32, kind="Internal")
    agout_d = nc.dram_tensor(
        "agout", [c.C * c.TROW, HID], F32, kind="Internal", addr_space="Shared")
    aggb_d = nc.dram_tensor("aggb", [AGGROWS, HID], F32, kind="Internal")
    plin_d = nc.dram_tensor("plin", [128, HID], F32, kind="Internal")
    plout_d = nc.dram_tensor(
        "plout", [128, HID], F32, kind="Internal", addr_space="Shared")
    y_d = nc.dram_tensor("y", [G, OUT], F32, kind="ExternalOutput")

    rg = [list(range(c.C))]
    NVTOT = AGGROWS - c.TROW            # dump + virtual rows region

    with tile.TileContext(nc) as tc:
        with (
            tc.tile_pool(name="res", bufs=1) as res,
            tc.tile_pool(name="stage", bufs=1) as stpool,
            tc.tile_pool(name="work", bufs=6) as work,
            tc.tile_pool(name="msgs", bufs=6) as msgs,
            tc.tile_pool(name="psA", bufs=2, space="PSUM") as psA,
            tc.tile_pool(name="psB", bufs=2, space="PSUM") as psB,
        ):
            ident = res.tile([128, 128], F32)
            make_identity(nc, ident[:])
            dinv_sb = res.tile([128, c.NCHK], F32)
            nc.sync.dma_start(dinv_sb[:], dinv_d[:])
            W_sb = [res.tile([c.IN, HID], F32, name="w1"),
                    res.tile([HID, HID], F32, name="w2"),
                    res.tile([HID, HID], F32, name="w3")]
            nc.sync.dma_start(W_sb[0][:], W1_d[:])
            nc.sync.dma_start(W_sb[1][:], W2_d[:])
            nc.sync.dma_start(W_sb[2][:], W3_d[:])
            bcols_sb = res.tile([HID, 3], F32)
            nc.sync.dma_start(bcols_sb[:], bcols_d[:])
            b3rep_sb = res.tile([128, HID], F32)
            nc.sync.dma_start(b3rep_sb[:], b3rep_d[:])
            Wc_sb = res.tile([HID, OUT], F32)
            nc.sync.dma_start(Wc_sb[:], Wc_d[:])
            bcrep_sb = res.tile([128, OUT], F32)
            nc.sync.dma_start(bcrep_sb[:], bcrep_d[:])
            cinv_sb = res.tile([128, 1], F32)
            nc.sync.dma_start(cinv_sb[:], cinv_d[:])
            zero_sb = res.tile([128, cdiv(NVTOT, 128), HID], F32)
            nc.vector.memset(zero_sb[:], 0.0)

            hT_sb = stpool.tile([HID, c.TROW], F32)
            stage_sb = stpool.tile([128, c.NCHK, HID], F32)
            h3_sb = stpool.tile([128, c.NCHK, HID], F32)

            agin_r = agin_d[:].rearrange("(k p) f -> p k f", p=128)
            aggb_r = aggb_d[: c.TROW, :].rearrange("(k p) f -> p k f", p=128)
            aggv_r = aggb_d[c.TROW:, :].rearrange("(k p) f -> p k f", p=128)

            nreg = nc.gpsimd.alloc_register("nidx")
            _regval = [None]

            def set_nreg(v):
                if _regval[0] != v:
                    nc.gpsimd.reg_mov(nreg, v)
                    _regval[0] = v

            for l in range(3):
                K = c.IN if l == 0 else HID
                for k in range(c.NCHK):
                    if l == 0:
                        xt = work.tile([c.IN, 128], F32, tag="xt")
                        nc.sync.dma_start(xt[:], xT_d[:, k * 128:(k + 1) * 128])
                        lhsT = xt[:, :]
                    else:
                        lhsT = hT_sb[:K, k * 128:(k + 1) * 128]
                    ps = psA.tile([128, HID], F32, space="PSUM")
                    nc.tensor.matmul(ps[:], lhsT, W_sb[l][:K, :],
                                     start=True, stop=True)
                    nc.vector.tensor_scalar_mul(
                        stage_sb[:, k, :], ps[:], dinv_sb[:, k:k + 1])
                BK = 8
                for kk in range(cdiv(c.NCHK, BK)):
                    s = kk * BK
                    e = min(c.NCHK, s + BK)
                    nc.sync.dma_start(agin_r[:, s:e, :], stage_sb[:, s:e, :])
                    nc.sync.dma_start(aggb_r[:, s:e, :], stage_sb[:, s:e, :])
                # zero dump+virtual region
                nc.sync.dma_start(aggv_r[:], zero_sb[:])
                nc.gpsimd.collective_compute(
                    "AllGather", ALU.bypass,
                    replica_groups=rg, ins=[agin_d[:]], outs=[agout_d[:]])

                # software-pipelined: emit bin k+1's gathers before bin k's
                # scatter so the Pool engine never stalls on a gather DMA.
                pending = None  # (bin_sz, scol, msg)

                def flush_pending():
                    nonlocal pending
                    if pending is None:
                        return
                    p_sz, p_scol, p_msg = pending
                    si = work.tile([128, MAXMSG // 16], I16, tag="si",
                                   name=f"si_{l}_{p_scol}")
                    nc.sync.dma_start(
                        si[:, : p_sz // 16],
                        sidx_d[:, p_scol // 16:(p_scol + p_sz) // 16])
                    set_nreg(p_sz)
                    nc.gpsimd.dma_scatter_add(
                        aggb_d[:], p_msg[:, : p_sz // 128, :],
                        si[:, : p_sz // 16], p_sz, nreg, HID)
                    pending = None

                for bin_sz, scol, segs in CALLS:
                    msg = msgs.tile([128, MAXMSG // 128, HID], F32, tag="msg",
                                    name=f"msg_{l}_{scol}")
                    off = 0
                    for b, seg, col in segs:
                        gi = work.tile([128, MAXMSG // 16], I16, tag="gi",
                                       name=f"gi_{l}_{col}")
                        nc.sync.dma_start(
                            gi[:, : seg // 16],
                            gidx_d[:, col // 16:(col + seg) // 16])
                        set_nreg(seg)
                        nc.gpsimd.dma_gather(
                            msg[:, off // 128:(off + seg) // 128, :],
                            agout_d[b * c.SRCW:(b + 1) * c.SRCW, :],
                            gi[:, : seg // 16], seg, nreg, HID)
                        off += seg
                    flush_pending()
                    pending = (bin_sz, scol, msg)
                flush_pending()
                # merge virtual rows, deepest level first (reads aggb, so the
                # gather must follow all round scatters; keep these serial)
                col = TOT
                for vm in VM:
                    msg = msgs.tile([128, MAXMSG // 128, HID], F32, tag="msg",
                                    name=f"msgm_{l}_{col}")
                    gi = work.tile([128, MAXMSG // 16], I16, tag="gi",
                                   name=f"gim_{l}_{col}")
                    nc.sync.dma_start(
                        gi[:, : vm // 16], gidx_d[:, col // 16:(col + vm) // 16])
                    si = work.tile([128, MAXMSG // 16], I16, tag="si",
                                   name=f"sim_{l}_{col}")
                    nc.sync.dma_start(
                        si[:, : vm // 16], sidx_d[:, col // 16:(col + vm) // 16])
                    set_nreg(vm)
                    nc.gpsimd.dma_gather(
                        msg[:, : vm // 128, :], aggb_d[:],
                        gi[:, : vm // 16], vm, nreg, HID)
                    nc.gpsimd.dma_scatter_add(
                        aggb_d[:], msg[:, : vm // 128, :],
                        si[:, : vm // 16], vm, nreg, HID)
                    col += vm

                for kk in range(cdiv(c.NCHK, BK)):
                    s = kk * BK
                    e = min(c.NCHK, s + BK)
                    at = work.tile([128, BK, HID], F32, tag="at")
                    nc.sync.dma_start(at[:, : e - s, :], aggb_r[:, s:e, :])
                    for k in range(s, e):
                        v = work.tile([128, HID], F32, tag="v")
                        nc.vector.tensor_scalar_mul(
                            v[:], at[:, k - s, :], dinv_sb[:, k:k + 1])
                        if l < 2:
                            ps = psB.tile([HID, 128], F32, space="PSUM")
                            nc.tensor.transpose(ps[:], v[:], ident[:])
                            nc.scalar.activation(
                                hT_sb[:, k * 128:(k + 1) * 128], ps[:],
                                AF.Relu, bias=bcols_sb[:, l:l + 1])
                        else:
                            vb = work.tile([128, HID], F32, tag="vb")
                            nc.vector.tensor_add(vb[:], v[:], b3rep_sb[:])
                            nc.vector.tensor_relu(h3_sb[:, k, :], vb[:])

            # ---- mean pool ----
            pp = psA.tile([128, HID], F32, space="PSUM", tag="pool", bufs=1)
            for k in range(c.NCHK):
                oh = work.tile([128, 128], F32, tag="oh")
                nc.sync.dma_start(oh[:], oneh_d[k * 128:(k + 1) * 128, :])
                nc.tensor.matmul(
                    pp[:], oh[:], h3_sb[:, k, :],
                    start=(k == 0), stop=(k == c.NCHK - 1))
            pl = res.tile([128, HID], F32)
            nc.vector.tensor_copy(pl[:], pp[:])
            nc.sync.dma_start(plin_d[:], pl[:])
            nc.gpsimd.collective_compute(
                "AllReduce", ALU.add,
                replica_groups=rg, ins=[plin_d[:]], outs=[plout_d[:]])
            plr = res.tile([128, HID], F32)
            nc.sync.dma_start(plr[:], plout_d[:])
            plm = res.tile([128, HID], F32)
            nc.vector.tensor_scalar_mul(plm[:], plr[:], cinv_sb[:])
            psT = psB.tile([HID, 128], F32, space="PSUM", tag="pT", bufs=1)
            nc.tensor.transpose(psT[:], plm[:], ident[:])
            plT = res.tile([HID, 128], F32)
            nc.vector.tensor_copy(plT[:], psT[:])
            psC = psB.tile([G, OUT], F32, space="PSUM", tag="pC", bufs=1)
            nc.tensor.matmul(psC[:], plT[:, :G], Wc_sb[:], start=True, stop=True)
            lg = res.tile([G, OUT], F32)
            nc.vector.tensor_add(lg[:], psC[:, :], bcrep_sb[:G, :])
            mx = res.tile([G, 1], F32)
            nc.vector.tensor_reduce(mx[:], lg[:], mybir.AxisListType.X, ALU.max)
            lgs = res.tile([G, OUT], F32)
            nc.vector.tensor_scalar_sub(lgs[:], lg[:], mx[:])
            ex = res.tile([G, OUT], F32)
            nc.scalar.activation(ex[:], lgs[:], AF.Exp)
            sm = res.tile([G, 1], F32)
            nc.vector.tensor_reduce(sm[:], ex[:], mybir.AxisListType.X, ALU.add)
            ls = res.tile([G, 1], F32)
            nc.scalar.activation(ls[:], sm[:], AF.Ln)
            yt = res.tile([G, OUT], F32)
            nc.vector.tensor_scalar_sub(yt[:], lgs[:], ls[:])
            nc.sync.dma_start(y_d[:], yt[:])

    return nc


def _finalize(nc):
    nc.compile()
    fix_multiwait(nc)


def run(inputs, cfg, profile_dir=None):
    from concourse.bass_utils import run_bass_kernel_spmd

    in_maps, meta = prep(inputs, cfg)
    nc = build(cfg, meta)
    _finalize(nc)
    if profile_dir is not None:
        from trn_agent_boot.trn_boot import _ntff_profile_via_ctypes
        hook = _ntff_profile_via_ctypes("/opt/axon/libaxon_pjrt.so")
        with hook(profile_dir, [0]):
            res = run_bass_kernel_spmd(nc, in_maps, core_ids=list(range(cfg.C)))
    else:
        res = run_bass_kernel_spmd(nc, in_maps, core_ids=list(range(cfg.C)))
    return res.results[0]["y"]
# ---------------------------------------------------------------------------
N_NODES, N_EDGES, IN_DIM, HID_DIM, N_GRAPHS, OUT_DIM = 100_000, 1_600_000, 128, 64, 128, 3


def kernel(**inputs):
    import os
    cfg = Cfg(N_NODES, N_EDGES, IN_DIM, HID_DIM, N_GRAPHS, OUT_DIM, LV=20)
    out = run(inputs, cfg, profile_dir=os.environ.get("GNN_PROFILE_DIR"))
    return np.asarray(out, np.float32)



# revision 5
# speedup vs baseline: 1.9336x; 1.9336x over previous
"""3-layer GCN + mean-pool + classifier for Trainium2, SPMD on 8 NeuronCores.

Self-contained: kernel(**inputs) takes the full-size numpy inputs, does the
host-side graph partitioning, builds/compiles a Bass/Tile kernel, runs it on
cores 0-7 via run_bass_kernel_spmd, and returns the [128, 3] log-softmax
output.

Distribution: nodes are dst-sharded across the 8 cores. Per GCN layer each
core computes t' = dinv * (h @ W) for its shard (TensorE), stores it as
256B bf16 rows (64 feats + 64 zero pad), AllGathers the shards into a full
node-major table in DRAM, and dma_gathers its in-edges' source rows. The
scatter-add of the previous design is replaced by TensorE accumulation:
edges are sorted by (dst-slab, src-bucket, dst), segments are padded to the
cross-core max so the call/window/run structure is SPMD-static, and each
128-slot window is reduced into per-chunk PSUM accumulators with one-hot
lhsT matrices built on DVE (iota==runvec compare). Self-loops never touch
the edge path: the t' staging tile is added to the PSUM result directly.
The GCN normalization deg^-1/2 (A+I) deg^-1/2 factorizes into a pre-scale
of t' and a post-scale of the aggregate. Mean-pooling runs as a one-hot
matmul on TensorE with an AllReduce of per-core partials; the classifier +
log_softmax run replicated on every core.

HW limits (empirical): gather idx values must be < 8192 (16 source
buckets), gather calls <= 1024 indices. Pad gather slots use index 0 (mid-
call -1 is unsafe); their one-hot rows are 255 so they contribute zero.
"""
import sys

sys.path.insert(0, "/opt/trn_rl_repo")

import numpy as np
import ml_dtypes
import concourse.bacc as bacc
import concourse.mybir as mybir
import concourse.tile as tile
from concourse.masks import make_identity
import concourse.tile as _tile
import concourse.mybir as _mybir
from concourse.vector_clock import ScopedClock as _ScopedClock

# ---------------------------------------------------------------------------
# Workarounds: this walrus build rejects >1 sync-wait per instruction.


def _split_waits_tail(nc, inst):
    si = inst.ins.sync_info
    if si is None or not si.on_wait or len(si.on_wait) <= 1:
        return
    waits = list(si.on_wait)
    inst.ins.sync_info = _mybir.SyncInfo(on_wait=[], on_update=list(si.on_update or []))
    for w in waits:
        nop = nc.sync.nop()
        nop.ins.sync_info = _mybir.SyncInfo(on_wait=[w], on_update=[])


def _drain_and_barrier(self, tick_clock, wait_clock):
    nc = self.nc
    probe = nc.sync.nop()
    wait_clock.add_sem_waits(probe.ins, _ScopedClock({None: tick_clock.global_clock}))
    _split_waits_tail(nc, probe)
    nc.sync.drain()
    nc.all_engine_barrier()
    assert self.sems is not None
    popped = nc._tile_sem_poison_stack.pop()
    assert popped is self._sem_poison
    nc.clear_and_free_semaphores(list(self.sems.allocated().values()))
    nc.all_engine_barrier()


_tile.TileContext._drain_and_barrier = _drain_and_barrier


def fix_multiwait(nc):
    """Rewrite every >1-wait instruction into wait-nops + 1-wait instruction."""
    for f in nc.m.functions:
        for blk in f.blocks:
            insts = blk.instructions            # live list (rust-backed)
            i = 0
            while i < len(insts):
                inst = insts[i]
                si = inst.sync_info
                if si is not None and si.on_wait and len(si.on_wait) > 1:
                    waits = list(si.on_wait)
                    eng = inst.engine
                    inst.sync_info = _mybir.SyncInfo(
                        on_wait=[waits[-1]], on_update=list(si.on_update or [])
                    )
                    for j, w in enumerate(waits[:-1]):
                        nop = nc.engines[eng].nop(hint="mwfix")
                        popped = False
                        for f2 in nc.m.functions:
                            for b2 in f2.blocks:
                                l2 = b2.instructions
                                if l2 and l2[-1].name == nop.ins.name:
                                    l2.pop()
                                    popped = True
                                    break
                            if popped:
                                break
                        assert popped, "could not relocate mwfix nop"
                        nop.ins.sync_info = _mybir.SyncInfo(on_wait=[w], on_update=[])
                        insts.insert(i + j, nop.ins)
                    i += len(waits) - 1
                i += 1


# ---------------------------------------------------------------------------

F32 = mybir.dt.float32
BF16 = mybir.dt.bfloat16
I16 = mybir.dt.int16
AF = mybir.ActivationFunctionType
ALU = mybir.AluOpType
BF16NP = ml_dtypes.bfloat16


def cdiv(a, b):
    return (a + b - 1) // b


def rup(a, b):
    return cdiv(a, b) * b


class Cfg:
    def __init__(self, N, E, IN, HID, G, OUT):
        self.C = 8
        self.N, self.E, self.IN, self.HID, self.G, self.OUT = N, E, IN, HID, G, OUT
        assert N % self.C == 0
        self.NSH = N // self.C            # 12500
        self.TROW = rup(self.NSH, 128)    # 12544
        self.NCHK = self.TROW // 128      # 98
        self.NBUK = 16
        assert (self.C * self.TROW) % self.NBUK == 0
        self.SRCW = self.C * self.TROW // self.NBUK   # 6272
        assert self.SRCW <= 8191
        self.S = 7                        # chunks per slab
        assert self.NCHK % self.S == 0
        self.NSLAB = self.NCHK // self.S  # 14
        self.EL = 128                     # bf16 elems per table row (256B)
        self.MAXG = 1024
        assert G <= 128


def prep(inputs, cfg):
    c = cfg
    x = np.asarray(inputs["x"], np.float32)
    ei = np.asarray(inputs["edge_index"], np.int64)
    batch = np.asarray(inputs["batch"], np.int64)
    W1 = np.asarray(inputs["W1"], np.float32); b1 = np.asarray(inputs["b1"], np.float32)
    W2 = np.asarray(inputs["W2"], np.float32); b2 = np.asarray(inputs["b2"], np.float32)
    W3 = np.asarray(inputs["W3"], np.float32); b3 = np.asarray(inputs["b3"], np.float32)
    Wc = np.asarray(inputs["Wc"], np.float32); bc = np.asarray(inputs["bc"], np.float32)

    src = ei[0].astype(np.int64)
    dst = ei[1].astype(np.int64)
    deg = np.bincount(dst, minlength=c.N).astype(np.float32) + 1.0
    dinv = 1.0 / np.sqrt(deg)

    HID = c.HID
    W3p = np.zeros((HID, HID), np.float32); W3p[:, : W3.shape[1]] = W3
    b3p = np.zeros((HID,), np.float32); b3p[: b3.shape[0]] = b3
    Wcp = np.zeros((HID, c.OUT), np.float32); Wcp[: Wc.shape[0]] = Wc

    core_of = src // c.NSH
    trow_src = core_of * c.TROW + (src - core_of * c.NSH)
    buk_all = (trow_src // c.SRCW).astype(np.int64)
    gidx_all = (trow_src - buk_all * c.SRCW).astype(np.int64)
    dcore = dst // c.NSH

    # Per-core edge lists grouped by (chunk, bucket), sorted by dst within.
    # seg_edges[ci][(chunk, bucket)] = (gather_idx array, dst_local array)
    seg_edges = []
    cnt = np.zeros((c.C, c.NCHK, c.NBUK), np.int64)
    for ci in range(c.C):
        m = dcore == ci
        dl = dst[m] - ci * c.NSH
        gg = gidx_all[m]
        bb = buk_all[m]
        ch = dl // 128
        order = np.lexsort((dl, bb, ch))
        dl, gg, bb, ch = dl[order], gg[order], bb[order], ch[order]
        key = ch * c.NBUK + bb
        d = {}
        bounds = np.r_[0, np.flatnonzero(np.diff(key)) + 1, len(key)]
        for i in range(len(bounds) - 1):
            a, b = bounds[i], bounds[i + 1]
            d[(int(ch[a]), int(bb[a]))] = (gg[a:b], dl[a:b])
            cnt[ci, ch[a], bb[a]] = b - a
        seg_edges.append(d)
    SEG = cnt.max(axis=0)  # [NCHK, NBUK] cross-core segment sizes

    # Static call/window/run layout.
    # calls: list of dicts(slab, bucket, n (x128), col (slot offset into gidx),
    #                     runs: [(window, chunk_or_-1, a, b, run_id, first, last)])
    calls = []
    run_chunk = []           # chunk of each run (global run id)
    slot_chunk_all = []      # per slot: (chunk or -1)
    col = 0
    nruns = 0
    first_seen = {}
    for s in range(c.NSLAB):
        chunks = list(range(s * c.S, (s + 1) * c.S))
        stream = []          # (chunk) per slot of this (slab,bucket) stream
        for b in range(c.NBUK):
            sb = []
            for ch in chunks:
                sb.extend([ch] * int(SEG[ch, b]))
            # split into calls of <= MAXG
            off = 0
            while off < len(sb):
                n_raw = min(c.MAXG, len(sb) - off)
                n = rup(n_raw, 128)
                chunk_of = sb[off:off + n_raw] + [-1] * (n - n_raw)
                # runs: maximal (window, chunk) groups
                runs = []
                for w in range(n // 128):
                    a = 0
                    wslots = chunk_of[w * 128:(w + 1) * 128]
                    while a < 128:
                        ch0 = wslots[a]
                        e = a
                        while e < 128 and wslots[e] == ch0:
                            e += 1
                        if ch0 >= 0:
                            runs.append([w, ch0, a, e, nruns, False, False])
                            run_chunk.append(ch0)
                            nruns += 1
                        a = e
                calls.append(dict(slab=s, bucket=b, n=n, col=col, runs=runs))
                slot_chunk_all.extend(chunk_of)
                col += n
                off += n_raw
    TOTSLOT = col
    # first/last flags per chunk
    last_seen = {}
    for call in calls:
        for r in call["runs"]:
            ch = r[1]
            if ch not in first_seen:
                first_seen[ch] = r
            last_seen[ch] = r
    for ch, r in first_seen.items():
        r[5] = True
    for ch, r in last_seen.items():
        r[6] = True
    assert len(first_seen) == c.NCHK, "every chunk must have runs"

    cntg = np.bincount(batch, minlength=c.G).astype(np.float32)
    cntinv = (1.0 / np.maximum(cntg, 1.0)).astype(np.float32)

    # Per-core tensors.
    in_maps = []
    for ci in range(c.C):
        lo, hi = ci * c.NSH, (ci + 1) * c.NSH
        xT = np.zeros((c.IN, c.TROW), np.float32)
        xT[:, : c.NSH] = x[lo:hi].T
        dv = np.zeros((c.TROW,), np.float32)
        dv[: c.NSH] = dinv[lo:hi]
        dinv2d = dv.reshape(c.NCHK, 128).T.copy()

        g_slots = np.zeros(TOTSLOT, np.int64)
        runvecs = np.full((128, max(nruns, 1)), 255.0, np.float32)
        segs = seg_edges[ci]
        # walk the same static layout, filling per-core gather idx + runvecs
        pos = {}   # (chunk,bucket) -> consumed count
        for call in calls:
            s, b, n, col0 = call["slab"], call["bucket"], call["n"], call["col"]
            # rebuild chunk_of for this call from runs is lossy (pads) — use
            # global slot_chunk_all
            chunk_of = slot_chunk_all[col0:col0 + n]
            for i in range(n):
                ch = chunk_of[i]
                if ch < 0:
                    continue
                k = pos.get((ch, b), 0)
                ge, de = segs.get((ch, b), (None, None))
                if ge is not None and k < len(ge):
                    g_slots[col0 + i] = ge[k]
                    # which run does this slot belong to?
                pos[(ch, b)] = k + 1
            for w, ch, a, e, rid, _, _ in call["runs"]:
                pass
        # second pass: runvecs (needs per-slot real/pad + dst_local)
        pos = {}
        for call in calls:
            s, b, n, col0 = call["slab"], call["bucket"], call["n"], call["col"]
            chunk_of = slot_chunk_all[col0:col0 + n]
            # per-slot dst_local%128 or -1
            dloc = np.full(n, -1, np.int64)
            for i in range(n):
                ch = chunk_of[i]
                if ch < 0:
                    continue
                k = pos.get((ch, b), 0)
                ge, de = segs.get((ch, b), (None, None))
                if ge is not None and k < len(de):
                    dloc[i] = de[k] % 128
                pos[(ch, b)] = k + 1
            for w, ch, a, e, rid, _, _ in call["runs"]:
                sl = dloc[w * 128 + a: w * 128 + e]
                rv = np.full(e - a, 255.0, np.float32)
                rv[sl >= 0] = sl[sl >= 0].astype(np.float32)
                runvecs[a:e, rid] = rv
        gidx_w = np.tile(
            g_slots.astype(np.int16).reshape(-1, 16).T, (8, 1)).astype(np.int16)

        oneh = np.zeros((c.TROW, 128), np.float32)
        oneh[np.arange(c.NSH), batch[lo:hi].astype(np.int64)] = 1.0

        bcols = np.stack([b1, b2, b3p], axis=1)
        b3rep = np.tile(b3p[None, :], (128, 1))
        bcrep = np.tile(bc[None, :], (128, 1))
        cinv = np.zeros((128, 1), np.float32)
        cinv[: c.G, 0] = cntinv
        iota = np.tile(np.arange(128, dtype=np.float32), (128, 1))

        in_maps.append(dict(
            xT=xT, dinv2d=dinv2d, gidx=gidx_w, runvecs=runvecs,
            oneh=oneh.astype(BF16NP),
            W1d=W1, W2d=W2.astype(BF16NP), W3d=W3p.astype(BF16NP),
            bcols=bcols, b3rep=b3rep, Wcp=Wcp, bcrep=bcrep, cinv=cinv,
            iota=iota,
        ))

    meta = dict(calls=calls, nruns=nruns, TOTSLOT=TOTSLOT)
    return in_maps, meta


def build(cfg, meta):
    c = cfg
    HID, G, OUT, EL = c.HID, c.G, c.OUT, c.EL
    calls, NRUNS, TOTSLOT = meta["calls"], meta["nruns"], meta["TOTSLOT"]

    nc = bacc.Bacc("TRN2", num_devices=c.C, dynamic_dma_scratch_size=65536)

    def ein(name, shape, dt=F32):
        return nc.dram_tensor(name, shape, dt, kind="ExternalInput")

    xT_d = ein("xT", [c.IN, c.TROW])
    dinv_d = ein("dinv2d", [128, c.NCHK])
    gidx_d = ein("gidx", [128, TOTSLOT // 16], I16)
    runv_d = ein("runvecs", [128, NRUNS])
    oneh_d = ein("oneh", [c.TROW, 128], BF16)
    W1_d = ein("W1d", [c.IN, HID])
    W2_d = ein("W2d", [HID, HID], BF16)
    W3_d = ein("W3d", [HID, HID], BF16)
    bcols_d = ein("bcols", [HID, 3])
    b3rep_d = ein("b3rep", [128, HID])
    Wc_d = ein("Wcp", [HID, OUT])
    bcrep_d = ein("bcrep", [128, OUT])
    cinv_d = ein("cinv", [128, 1])
    iota_d = ein("iota", [128, 128])

    agin_d = nc.dram_tensor("agin", [c.TROW, EL], BF16, kind="Internal")
    agout_d = nc.dram_tensor(
        "agout", [c.C * c.TROW, EL], BF16, kind="Internal", addr_space="Shared")
    plin_d = nc.dram_tensor("plin", [128, HID], F32, kind="Internal")
    plout_d = nc.dram_tensor(
        "plout", [128, HID], F32, kind="Internal", addr_space="Shared")
    y_d = nc.dram_tensor("y", [G, OUT], F32, kind="ExternalOutput")

    rg = [list(range(c.C))]

    with tile.TileContext(nc) as tc:
        with (
            tc.tile_pool(name="res", bufs=1) as res,
            tc.tile_pool(name="stage", bufs=1) as stpool,
            tc.tile_pool(name="work", bufs=8) as work,
            tc.tile_pool(name="ohp", bufs=8) as ohp,
            tc.tile_pool(name="msgs", bufs=6) as msgs,
            tc.tile_pool(name="psA", bufs=2, space="PSUM") as psA,
            tc.tile_pool(name="psB", bufs=1, space="PSUM") as psB,
            tc.tile_pool(name="psC", bufs=2, space="PSUM") as psC,
        ):
            ident = res.tile([128, 128], F32)
            make_identity(nc, ident[:])
            dinv_sb = res.tile([128, c.NCHK], F32)
            nc.sync.dma_start(dinv_sb[:], dinv_d[:])
            W1_sb = res.tile([c.IN, HID], F32, name="w1")
            nc.sync.dma_start(W1_sb[:], W1_d[:])
            W2_sb = res.tile([HID, HID], BF16, name="w2")
            nc.sync.dma_start(W2_sb[:], W2_d[:])
            W3_sb = res.tile([HID, HID], BF16, name="w3")
            nc.sync.dma_start(W3_sb[:], W3_d[:])
            bcols_sb = res.tile([HID, 3], F32)
            nc.sync.dma_start(bcols_sb[:], bcols_d[:])
            b3rep_sb = res.tile([128, HID], F32)
            nc.sync.dma_start(b3rep_sb[:], b3rep_d[:])
            Wc_sb = res.tile([HID, OUT], F32)
            nc.sync.dma_start(Wc_sb[:], Wc_d[:])
            bcrep_sb = res.tile([128, OUT], F32)
            nc.sync.dma_start(bcrep_sb[:], bcrep_d[:])
            cinv_sb = res.tile([128, 1], F32)
            nc.sync.dma_start(cinv_sb[:], cinv_d[:])
            iota_sb = res.tile([128, 128], F32)
            nc.sync.dma_start(iota_sb[:], iota_d[:])
            runv_sb = res.tile([128, NRUNS], F32)
            nc.sync.dma_start(runv_sb[:], runv_d[:])

            hT_sb = stpool.tile([HID, c.TROW], BF16)
            stage_sb = stpool.tile([128, c.NCHK, HID], F32)
            stagebf_sb = stpool.tile([128, c.NCHK, EL], BF16)
            nc.vector.memset(stagebf_sb[:], 0.0)
            h3_sb = stpool.tile([128, c.NCHK, HID], BF16)

            agin_r = agin_d[:].rearrange("(k p) f -> p k f", p=128)

            nreg = nc.gpsimd.alloc_register("nidx")
            _regval = [None]

            def set_nreg(v):
                if _regval[0] != v:
                    nc.gpsimd.reg_mov(nreg, v)
                    _regval[0] = v

            for l in range(3):
                K = c.IN if l == 0 else HID
                W_sb = [W1_sb, W2_sb, W3_sb][l]
                # ---- phase 1: t' = dinv * (h @ W), staged f32 + bf16 ----
                for k in range(c.NCHK):
                    if l == 0:
                        xt = work.tile([c.IN, 128], F32, tag="xt")
                        nc.sync.dma_start(xt[:], xT_d[:, k * 128:(k + 1) * 128])
                        lhsT = xt[:, :]
                    else:
                        lhsT = hT_sb[:K, k * 128:(k + 1) * 128]
                    ps = psA.tile([128, HID], F32, space="PSUM")
                    nc.tensor.matmul(ps[:], lhsT, W_sb[:K, :],
                                     start=True, stop=True)
                    nc.vector.tensor_scalar_mul(
                        stage_sb[:, k, :], ps[:], dinv_sb[:, k:k + 1])
                    nc.vector.tensor_copy(
                        stagebf_sb[:, k, :HID], stage_sb[:, k, :])
                BK = 8
                for kk in range(cdiv(c.NCHK, BK)):
                    s0 = kk * BK
                    e0 = min(c.NCHK, s0 + BK)
                    nc.sync.dma_start(
                        agin_r[:, s0:e0, :], stagebf_sb[:, s0:e0, :])
                nc.gpsimd.collective_compute(
                    "AllGather", ALU.bypass,
                    replica_groups=rg, ins=[agin_d[:]], outs=[agout_d[:]])

                # ---- phase 2: edge pass, slab by slab ----
                pchunk = {}
                slab_ps = [None]
                cur_slab = -1

                def flush_slab(slab):
                    for ch in sorted(pchunk):
                        pc = pchunk[ch]
                        v = work.tile([128, HID], F32, tag="v")
                        nc.vector.tensor_tensor(
                            v[:], pc, stage_sb[:, ch, :], ALU.add)
                        v2 = work.tile([128, HID], F32, tag="v2")
                        nc.vector.tensor_scalar_mul(
                            v2[:], v[:], dinv_sb[:, ch:ch + 1])
                        if l < 2:
                            psT = psB.tile([HID, 128], F32, space="PSUM")
                            nc.tensor.transpose(psT[:], v2[:], ident[:])
                            nc.scalar.activation(
                                hT_sb[:, ch * 128:(ch + 1) * 128], psT[:],
                                AF.Relu, bias=bcols_sb[:, l:l + 1])
                        else:
                            vb = work.tile([128, HID], F32, tag="vb")
                            nc.vector.tensor_add(vb[:], v2[:], b3rep_sb[:])
                            nc.vector.tensor_relu(h3_sb[:, ch, :], vb[:])
                    pchunk.clear()

                for call in calls:
                    s, b, n, col0 = (call["slab"], call["bucket"],
                                     call["n"], call["col"])
                    if s != cur_slab:
                        flush_slab(cur_slab)
                        slab_ps[0] = None
                        cur_slab = s
                    gi = work.tile([128, c.MAXG // 16], I16, tag="gi",
                                   name=f"gi_{l}_{col0}")
                    nc.sync.dma_start(
                        gi[:, : n // 16],
                        gidx_d[:, col0 // 16:(col0 + n) // 16])
                    msg = msgs.tile([128, c.MAXG // 128, EL], BF16, tag="msg",
                                    name=f"msg_{l}_{col0}")
                    set_nreg(n)
                    nc.gpsimd.dma_gather(
                        msg[:, : n // 128, :],
                        agout_d[b * c.SRCW:(b + 1) * c.SRCW, :],
                        gi[:, : n // 16], n, nreg, EL)
                    for w, ch, a, e, rid, first, lastf in call["runs"]:
                        oh = ohp.tile([128, 128], BF16, tag="oh",
                                      name=f"oh_{l}_{rid}")
                        nc.vector.tensor_scalar(
                            oh[:], iota_sb[:], runv_sb[:, rid:rid + 1], None,
                            ALU.is_equal)
                        if first:
                            assert ch not in pchunk
                            if slab_ps[0] is None:
                                slab_ps[0] = psC.tile(
                                    [128, c.S * HID], F32, space="PSUM",
                                    tag="pc", name=f"pc_{l}_{s}")
                            cc = ch - s * c.S
                            pchunk[ch] = slab_ps[0][:, cc * HID:(cc + 1) * HID]
                        nc.tensor.matmul(
                            pchunk[ch], oh[:], msg[:, w, :HID],
                            start=first, stop=lastf)
                flush_slab(cur_slab)

            # ---- mean pool ----
            pp = psA.tile([128, HID], F32, space="PSUM", tag="pool", bufs=1)
            for k in range(c.NCHK):
                oh = work.tile([128, 128], BF16, tag="oh2")
                nc.sync.dma_start(oh[:], oneh_d[k * 128:(k + 1) * 128, :])
                nc.tensor.matmul(
                    pp[:], oh[:], h3_sb[:, k, :],
                    start=(k == 0), stop=(k == c.NCHK - 1))
            pl = res.tile([128, HID], F32)
            nc.vector.tensor_copy(pl[:], pp[:])
            nc.sync.dma_start(plin_d[:], pl[:])
            nc.gpsimd.collective_compute(
                "AllReduce", ALU.add,
                replica_groups=rg, ins=[plin_d[:]], outs=[plout_d[:]])
            plr = res.tile([128, HID], F32)
            nc.sync.dma_start(plr[:], plout_d[:])
            plm = res.tile([128, HID], F32)
            nc.vector.tensor_scalar_mul(plm[:], plr[:], cinv_sb[:])
            psT = psB.tile([HID, 128], F32, space="PSUM", tag="pT", bufs=1)
            nc.tensor.transpose(psT[:], plm[:], ident[:])
            plT = res.tile([HID, 128], F32)
            nc.vector.tensor_copy(plT[:], psT[:])
            psD = psB.tile([G, OUT], F32, space="PSUM", tag="pC", bufs=1)
            nc.tensor.matmul(psD[:], plT[:, :G], Wc_sb[:], start=True, stop=True)
            lg = res.tile([G, OUT], F32)
            nc.vector.tensor_add(lg[:], psD[:, :], bcrep_sb[:G, :])
            mx = res.tile([G, 1], F32)
            nc.vector.tensor_reduce(mx[:], lg[:], mybir.AxisListType.X, ALU.max)
            lgs = res.tile([G, OUT], F32)
            nc.vector.tensor_scalar_sub(lgs[:], lg[:], mx[:])
            ex = res.tile([G, OUT], F32)
            nc.scalar.activation(ex[:], lgs[:], AF.Exp)
            sm = res.tile([G, 1], F32)
            nc.vector.tensor_reduce(sm[:], ex[:], mybir.AxisListType.X, ALU.add)
            ls = res.tile([G, 1], F32)
            nc.scalar.activation(ls[:], sm[:], AF.Ln)
            yt = res.tile([G, OUT], F32)
            nc.vector.tensor_scalar_sub(yt[:], lgs[:], ls[:])
            nc.sync.dma_start(y_d[:], yt[:])

    return nc


def _finalize(nc):
    nc.compile()
    fix_multiwait(nc)


def run(inputs, cfg, profile_dir=None):
    from concourse.bass_utils import run_bass_kernel_spmd

    in_maps, meta = prep(inputs, cfg)
    nc = build(cfg, meta)
    _finalize(nc)
    if profile_dir is not None:
        from trn_agent_boot.trn_boot import _ntff_profile_via_ctypes
        hook = _ntff_profile_via_ctypes("/opt/axon/libaxon_pjrt.so")
        with hook(profile_dir, [0]):
            res = run_bass_kernel_spmd(nc, in_maps, core_ids=list(range(cfg.C)))
    else:
        res = run_bass_kernel_spmd(nc, in_maps, core_ids=list(range(cfg.C)))
    return res.results[0]["y"]


# ---------------------------------------------------------------------------
N_NODES, N_EDGES, IN_DIM, HID_DIM, N_GRAPHS, OUT_DIM = 100_000, 1_600_000, 128, 64, 128, 3


def kernel(**inputs):
    import os
    cfg = Cfg(N_NODES, N_EDGES, IN_DIM, HID_DIM, N_GRAPHS, OUT_DIM)
    out = run(inputs, cfg, profile_dir=os.environ.get("GNN_PROFILE_DIR"))
    return np.asarray(out, np.float32)


# revision 12
# speedup vs baseline: 2.0017x; 1.0352x over previous
"""3-layer GCN + mean-pool + classifier for Trainium2, SPMD on 8 NeuronCores.

Self-contained: kernel(**inputs) takes the full-size numpy inputs, does the
host-side graph partitioning, builds/compiles a Bass/Tile kernel, runs it on
cores 0-7 via run_bass_kernel_spmd, and returns the [128, 3] log-softmax
output.

Distribution: nodes are dst-sharded across the 8 cores. Per GCN layer each
core computes t' = dinv * (h @ W) for its shard (TensorE), stores it as
256B bf16 rows (64 feats + 64 zero pad), AllGathers the shards into a full
node-major table in DRAM, and dma_gathers its in-edges' source rows. The
scatter-add of the previous design is replaced by TensorE accumulation:
edges are sorted by (dst-slab, src-bucket, dst), segments are padded to the
cross-core max so the call/window/run structure is SPMD-static, and each
128-slot window is reduced into per-chunk PSUM accumulators with one-hot
lhsT matrices built on DVE (iota==runvec compare). Self-loops never touch
the edge path: the t' staging tile is added to the PSUM result directly.
The GCN normalization deg^-1/2 (A+I) deg^-1/2 factorizes into a pre-scale
of t' and a post-scale of the aggregate. Mean-pooling runs as a one-hot
matmul on TensorE with an AllReduce of per-core partials; the classifier +
log_softmax run replicated on every core.

HW limits (empirical): gather idx values must be < 8192 (16 source
buckets), gather calls <= 1024 indices. Pad gather slots use index 0 (mid-
call -1 is unsafe); their one-hot rows are 255 so they contribute zero.
"""
import sys

sys.path.insert(0, "/opt/trn_rl_repo")

import numpy as np
import ml_dtypes
import concourse.bacc as bacc
import concourse.mybir as mybir
import concourse.tile as tile
from concourse.masks import make_identity
import concourse.tile as _tile
import concourse.mybir as _mybir
from concourse.vector_clock import ScopedClock as _ScopedClock

# ---------------------------------------------------------------------------
# Workarounds: this walrus build rejects >1 sync-wait per instruction.


def _split_waits_tail(nc, inst):
    si = inst.ins.sync_info
    if si is None or not si.on_wait or len(si.on_wait) <= 1:
        return
    waits = list(si.on_wait)
    inst.ins.sync_info = _mybir.SyncInfo(on_wait=[], on_update=list(si.on_update or []))
    for w in waits:
        nop = nc.sync.nop()
        nop.ins.sync_info = _mybir.SyncInfo(on_wait=[w], on_update=[])


def _drain_and_barrier(self, tick_clock, wait_clock):
    nc = self.nc
    probe = nc.sync.nop()
    wait_clock.add_sem_waits(probe.ins, _ScopedClock({None: tick_clock.global_clock}))
    _split_waits_tail(nc, probe)
    nc.sync.drain()
    nc.all_engine_barrier()
    assert self.sems is not None
    popped = nc._tile_sem_poison_stack.pop()
    assert popped is self._sem_poison
    nc.clear_and_free_semaphores(list(self.sems.allocated().values()))
    nc.all_engine_barrier()


_tile.TileContext._drain_and_barrier = _drain_and_barrier


def fix_multiwait(nc):
    """Rewrite every >1-wait instruction into wait-nops + 1-wait instruction."""
    for f in nc.m.functions:
        for blk in f.blocks:
            insts = blk.instructions            # live list (rust-backed)
            i = 0
            while i < len(insts):
                inst = insts[i]
                si = inst.sync_info
                if si is not None and si.on_wait and len(si.on_wait) > 1:
                    waits = list(si.on_wait)
                    eng = inst.engine
                    inst.sync_info = _mybir.SyncInfo(
                        on_wait=[waits[-1]], on_update=list(si.on_update or [])
                    )
                    for j, w in enumerate(waits[:-1]):
                        nop = nc.engines[eng].nop(hint="mwfix")
                        popped = False
                        for f2 in nc.m.functions:
                            for b2 in f2.blocks:
                                l2 = b2.instructions
                                if l2 and l2[-1].name == nop.ins.name:
                                    l2.pop()
                                    popped = True
                                    break
                            if popped:
                                break
                        assert popped, "could not relocate mwfix nop"
                        nop.ins.sync_info = _mybir.SyncInfo(on_wait=[w], on_update=[])
                        insts.insert(i + j, nop.ins)
                    i += len(waits) - 1
                i += 1


# ---------------------------------------------------------------------------

F32 = mybir.dt.float32
BF16 = mybir.dt.bfloat16
I16 = mybir.dt.int16
AF = mybir.ActivationFunctionType
ALU = mybir.AluOpType
BF16NP = ml_dtypes.bfloat16


def cdiv(a, b):
    return (a + b - 1) // b


def rup(a, b):
    return cdiv(a, b) * b


class Cfg:
    def __init__(self, N, E, IN, HID, G, OUT):
        self.C = 8
        self.N, self.E, self.IN, self.HID, self.G, self.OUT = N, E, IN, HID, G, OUT
        assert N % self.C == 0
        self.NSH = N // self.C            # 12500
        self.TROW = rup(self.NSH, 128)    # 12544
        self.NCHK = self.TROW // 128      # 98
        self.NBUK = 16
        assert (self.C * self.TROW) % self.NBUK == 0
        self.SRCW = self.C * self.TROW // self.NBUK   # 6272
        assert self.SRCW <= 8191
        self.S = 7                        # chunks per slab
        assert self.NCHK % self.S == 0
        self.NSLAB = self.NCHK // self.S  # 14
        self.EL = 128                     # bf16 elems per table row (256B)
        self.MAXG = 1024
        assert G <= 128


def prep(inputs, cfg):
    c = cfg
    x = np.asarray(inputs["x"], np.float32)
    ei = np.asarray(inputs["edge_index"], np.int64)
    batch = np.asarray(inputs["batch"], np.int64)
    W1 = np.asarray(inputs["W1"], np.float32); b1 = np.asarray(inputs["b1"], np.float32)
    W2 = np.asarray(inputs["W2"], np.float32); b2 = np.asarray(inputs["b2"], np.float32)
    W3 = np.asarray(inputs["W3"], np.float32); b3 = np.asarray(inputs["b3"], np.float32)
    Wc = np.asarray(inputs["Wc"], np.float32); bc = np.asarray(inputs["bc"], np.float32)

    src = ei[0].astype(np.int64)
    dst = ei[1].astype(np.int64)
    deg = np.bincount(dst, minlength=c.N).astype(np.float32) + 1.0
    dinv = 1.0 / np.sqrt(deg)

    HID = c.HID
    W3p = np.zeros((HID, HID), np.float32); W3p[:, : W3.shape[1]] = W3
    b3p = np.zeros((HID,), np.float32); b3p[: b3.shape[0]] = b3
    Wcp = np.zeros((HID, c.OUT), np.float32); Wcp[: Wc.shape[0]] = Wc

    core_of = src // c.NSH
    trow_src = core_of * c.TROW + (src - core_of * c.NSH)
    buk_all = (trow_src // c.SRCW).astype(np.int64)
    gidx_all = (trow_src - buk_all * c.SRCW).astype(np.int64)
    dcore = dst // c.NSH

    # Per-core edge lists grouped by (chunk, bucket), sorted by dst within.
    # seg_edges[ci][(chunk, bucket)] = (gather_idx array, dst_local array)
    seg_edges = []
    cnt = np.zeros((c.C, c.NCHK, c.NBUK), np.int64)
    for ci in range(c.C):
        m = dcore == ci
        dl = dst[m] - ci * c.NSH
        gg = gidx_all[m]
        bb = buk_all[m]
        ch = dl // 128
        order = np.lexsort((dl, bb, ch))
        dl, gg, bb, ch = dl[order], gg[order], bb[order], ch[order]
        key = ch * c.NBUK + bb
        d = {}
        bounds = np.r_[0, np.flatnonzero(np.diff(key)) + 1, len(key)]
        for i in range(len(bounds) - 1):
            a, b = bounds[i], bounds[i + 1]
            d[(int(ch[a]), int(bb[a]))] = (gg[a:b], dl[a:b])
            cnt[ci, ch[a], bb[a]] = b - a
        seg_edges.append(d)
    SEG = cnt.max(axis=0)  # [NCHK, NBUK] cross-core segment sizes

    # Static call/window/run layout.
    # calls: list of dicts(slab, bucket, n (x128), col (slot offset into gidx),
    #                     runs: [(window, chunk_or_-1, a, b, run_id, first, last)])
    calls = []
    run_chunk = []           # chunk of each run (global run id)
    slot_chunk_all = []      # per slot: (chunk or -1)
    col = 0
    nruns = 0
    first_seen = {}
    for s in range(c.NSLAB):
        chunks = list(range(s * c.S, (s + 1) * c.S))
        stream = []          # (chunk) per slot of this (slab,bucket) stream
        for b in range(c.NBUK):
            sb = []
            for ch in chunks:
                sb.extend([ch] * int(SEG[ch, b]))
            # split into calls of <= MAXG
            off = 0
            while off < len(sb):
                n_raw = min(c.MAXG, len(sb) - off)
                n = rup(n_raw, 128)
                chunk_of = sb[off:off + n_raw] + [-1] * (n - n_raw)
                # runs: maximal (window, chunk) groups
                runs = []
                for w in range(n // 128):
                    a = 0
                    wslots = chunk_of[w * 128:(w + 1) * 128]
                    while a < 128:
                        ch0 = wslots[a]
                        e = a
                        while e < 128 and wslots[e] == ch0:
                            e += 1
                        if ch0 >= 0:
                            runs.append([w, ch0, a, e, nruns, False, False])
                            run_chunk.append(ch0)
                            nruns += 1
                        a = e
                calls.append(dict(slab=s, bucket=b, n=n, col=col, runs=runs))
                slot_chunk_all.extend(chunk_of)
                col += n
                off += n_raw
    TOTSLOT = col
    # first/last flags per chunk
    last_seen = {}
    for call in calls:
        for r in call["runs"]:
            ch = r[1]
            if ch not in first_seen:
                first_seen[ch] = r
            last_seen[ch] = r
    for ch, r in first_seen.items():
        r[5] = True
    for ch, r in last_seen.items():
        r[6] = True
    assert len(first_seen) == c.NCHK, "every chunk must have runs"

    cntg = np.bincount(batch, minlength=c.G).astype(np.float32)
    cntinv = (1.0 / np.maximum(cntg, 1.0)).astype(np.float32)

    # Per-core tensors.
    in_maps = []
    for ci in range(c.C):
        lo, hi = ci * c.NSH, (ci + 1) * c.NSH
        xT = np.zeros((c.IN, c.TROW), np.float32)
        xT[:, : c.NSH] = x[lo:hi].T
        dv = np.zeros((c.TROW,), np.float32)
        dv[: c.NSH] = dinv[lo:hi]
        dinv2d = dv.reshape(c.NCHK, 128).T.copy()

        g_slots = np.zeros(TOTSLOT, np.int64)
        runvecs = np.full((128, max(nruns, 1)), 255.0, np.float32)
        segs = seg_edges[ci]
        # walk the same static layout, filling per-core gather idx + runvecs
        pos = {}   # (chunk,bucket) -> consumed count
        for call in calls:
            s, b, n, col0 = call["slab"], call["bucket"], call["n"], call["col"]
            # rebuild chunk_of for this call from runs is lossy (pads) — use
            # global slot_chunk_all
            chunk_of = slot_chunk_all[col0:col0 + n]
            for i in range(n):
                ch = chunk_of[i]
                if ch < 0:
                    continue
                k = pos.get((ch, b), 0)
                ge, de = segs.get((ch, b), (None, None))
                if ge is not None and k < len(ge):
                    g_slots[col0 + i] = ge[k]
                    # which run does this slot belong to?
                pos[(ch, b)] = k + 1
            for w, ch, a, e, rid, _, _ in call["runs"]:
                pass
        # second pass: runvecs (needs per-slot real/pad + dst_local)
        pos = {}
        for call in calls:
            s, b, n, col0 = call["slab"], call["bucket"], call["n"], call["col"]
            chunk_of = slot_chunk_all[col0:col0 + n]
            # per-slot dst_local%128 or -1
            dloc = np.full(n, -1, np.int64)
            for i in range(n):
                ch = chunk_of[i]
                if ch < 0:
                    continue
                k = pos.get((ch, b), 0)
                ge, de = segs.get((ch, b), (None, None))
                if ge is not None and k < len(de):
                    dloc[i] = de[k] % 128
                pos[(ch, b)] = k + 1
            for w, ch, a, e, rid, _, _ in call["runs"]:
                sl = dloc[w * 128 + a: w * 128 + e]
                rv = np.full(e - a, 255.0, np.float32)
                rv[sl >= 0] = sl[sl >= 0].astype(np.float32)
                runvecs[a:e, rid] = rv
        gidx_w = np.tile(
            g_slots.astype(np.int16).reshape(-1, 16).T, (8, 1)).astype(np.int16)
        # one-hot lhsT per run: [NRUNS*128, 128] bf16, onehot[r][p, d] = (runvec[p, r] == d)
        oh_all = (runvecs.T[:, :, None] ==
                  np.arange(128, dtype=np.float32)[None, None, :])
        oh_all = oh_all.reshape(nruns * 128, 128).astype(BF16NP)
        # per-node multipliers replicated across the 64 features, [128, NCHK*64]
        dinvrep = np.repeat(dv.reshape(c.NCHK, 128).T[:, :, None], 64, axis=2
                            ).reshape(128, c.NCHK * 64)
        dinv2rep = (dinvrep * dinvrep).astype(np.float32)
        b3rep_w = np.tile(b3p[None, None, :], (128, c.NCHK, 1)).reshape(128, c.NCHK * 64)

        oneh = np.zeros((c.TROW, 128), np.float32)
        oneh[np.arange(c.NSH), batch[lo:hi].astype(np.int64)] = 1.0

        bcols = np.stack([b1, b2, b3p], axis=1)
        bcrep = np.tile(bc[None, :], (128, 1))
        cinv = np.zeros((128, 1), np.float32)
        cinv[: c.G, 0] = cntinv

        in_maps.append(dict(
            xT=xT, dinv2d=dinv2d, gidx=gidx_w, ohruns=oh_all,
            dinvrep=dinvrep.astype(BF16NP),
            b3repw=b3rep_w.astype(BF16NP),
            oneh=oneh.astype(BF16NP),
            W1d=W1, W2d=W2.astype(BF16NP), W3d=W3p.astype(BF16NP),
            bcols=bcols, Wcp=Wcp, bcrep=bcrep, cinv=cinv,
        ))

    maxruns = max((len(cl["runs"]) for cl in calls), default=1)
    meta = dict(calls=calls, nruns=nruns, TOTSLOT=TOTSLOT, MAXRUNS=maxruns)
    return in_maps, meta


def build(cfg, meta):
    c = cfg
    HID, G, OUT, EL = c.HID, c.G, c.OUT, c.EL
    calls, NRUNS, TOTSLOT = meta["calls"], meta["nruns"], meta["TOTSLOT"]
    MAXRUNS = meta["MAXRUNS"]

    nc = bacc.Bacc("TRN2", num_devices=c.C, dynamic_dma_scratch_size=16384)

    def ein(name, shape, dt=F32):
        return nc.dram_tensor(name, shape, dt, kind="ExternalInput")

    xT_d = ein("xT", [c.IN, c.TROW])
    dinv_d = ein("dinv2d", [128, c.NCHK])
    gidx_d = ein("gidx", [128, TOTSLOT // 16], I16)
    ohruns_d = ein("ohruns", [NRUNS * 128, 128], BF16)
    dinvrep_d = ein("dinvrep", [128, c.NCHK * HID], BF16)
    b3repw_d = ein("b3repw", [128, c.NCHK * HID], BF16)
    oneh_d = ein("oneh", [c.TROW, 128], BF16)
    W1_d = ein("W1d", [c.IN, HID])
    W2_d = ein("W2d", [HID, HID], BF16)
    W3_d = ein("W3d", [HID, HID], BF16)
    bcols_d = ein("bcols", [HID, 3])
    Wc_d = ein("Wcp", [HID, OUT])
    bcrep_d = ein("bcrep", [128, OUT])
    cinv_d = ein("cinv", [128, 1])

    agin_d = nc.dram_tensor("agin", [c.TROW, EL], BF16, kind="Internal")
    agout_d = nc.dram_tensor(
        "agout", [c.C * c.TROW, EL], BF16, kind="Internal", addr_space="Shared")
    plin_d = nc.dram_tensor("plin", [128, HID], F32, kind="Internal")
    plout_d = nc.dram_tensor(
        "plout", [128, HID], F32, kind="Internal", addr_space="Shared")
    y_d = nc.dram_tensor("y", [G, OUT], F32, kind="ExternalOutput")

    rg = [list(range(c.C))]

    with tile.TileContext(nc) as tc:
        with (
            tc.tile_pool(name="res", bufs=1) as res,
            tc.tile_pool(name="stage", bufs=1) as stpool,
            tc.tile_pool(name="work", bufs=8) as work,
            tc.tile_pool(name="ohp", bufs=8) as ohp,
            tc.tile_pool(name="msgs", bufs=6) as msgs,
            tc.tile_pool(name="psA", bufs=2, space="PSUM") as psA,
            tc.tile_pool(name="psB", bufs=1, space="PSUM") as psB,
            tc.tile_pool(name="psC", bufs=2, space="PSUM") as psC,
        ):
            ident = res.tile([128, 128], F32)
            make_identity(nc, ident[:])
            dinvrep_sb = res.tile([128, c.NCHK, HID], BF16)
            nc.sync.dma_start(
                dinvrep_sb[:].rearrange("p a b -> p (a b)"), dinvrep_d[:])
            b3repw_sb = res.tile([128, c.NCHK, HID], BF16)
            nc.sync.dma_start(
                b3repw_sb[:].rearrange("p a b -> p (a b)"), b3repw_d[:])
            W1_sb = res.tile([c.IN, HID], F32, name="w1")
            nc.sync.dma_start(W1_sb[:], W1_d[:])
            W2_sb = res.tile([HID, HID], BF16, name="w2")
            nc.sync.dma_start(W2_sb[:], W2_d[:])
            W3_sb = res.tile([HID, HID], BF16, name="w3")
            nc.sync.dma_start(W3_sb[:], W3_d[:])
            bcols_sb = res.tile([HID, 3], F32)
            nc.sync.dma_start(bcols_sb[:], bcols_d[:])
            Wc_sb = res.tile([HID, OUT], F32)
            nc.sync.dma_start(Wc_sb[:], Wc_d[:])
            bcrep_sb = res.tile([128, OUT], F32)
            nc.sync.dma_start(bcrep_sb[:], bcrep_d[:])
            cinv_sb = res.tile([128, 1], F32)
            nc.sync.dma_start(cinv_sb[:], cinv_d[:])

            hT_sb = stpool.tile([HID, c.TROW], BF16)
            stage2_sb = stpool.tile([128, c.NCHK, HID], F32)
            stagebf_sb = stpool.tile([128, c.NCHK, HID], BF16)
            h3_sb = stpool.tile([128, c.NCHK, HID], BF16)

            agin_r = agin_d[:].rearrange("(k p) f -> p k f", p=128)
            # zero agin's upper 64-col half once (table rows are 256B)
            nc.vector.memset(stagebf_sb[:], 0.0)
            nc.sync.dma_start(agin_r[:, :, HID:], stagebf_sb[:])

            nreg = nc.gpsimd.alloc_register("nidx")
            _regval = [None]

            def set_nreg(v):
                if _regval[0] != v:
                    nc.gpsimd.reg_mov(nreg, v)
                    _regval[0] = v

            OCT = 7
            for l in range(3):
                K = c.IN if l == 0 else HID
                W_sb = [W1_sb, W2_sb, W3_sb][l]
                # ---- phase 1: t' = dinv * (h @ W), oct-batched ----
                for ko in range(0, c.NCHK, OCT):
                    ke = min(c.NCHK, ko + OCT)
                    ps = psA.tile([128, OCT, HID], F32, space="PSUM")
                    for k in range(ko, ke):
                        if l == 0:
                            xt = work.tile([c.IN, 128], F32, tag="xt")
                            nc.sync.dma_start(
                                xt[:], xT_d[:, k * 128:(k + 1) * 128])
                            lhsT = xt[:, :]
                        else:
                            lhsT = hT_sb[:K, k * 128:(k + 1) * 128]
                        nc.tensor.matmul(ps[:, k - ko, :], lhsT, W_sb[:K, :],
                                         start=True, stop=True)
                    nb = ke - ko
                    nc.vector.tensor_tensor(
                        stagebf_sb[:, ko:ke, :], ps[:, :nb, :],
                        dinvrep_sb[:, ko:ke, :], ALU.mult)
                    nc.vector.tensor_tensor(
                        stage2_sb[:, ko:ke, :], ps[:, :nb, :],
                        dinvrep_sb[:, ko:ke, :], ALU.mult)
                    nc.vector.tensor_tensor(
                        stage2_sb[:, ko:ke, :], stage2_sb[:, ko:ke, :],
                        dinvrep_sb[:, ko:ke, :], ALU.mult)
                    if l == 2:
                        nc.vector.tensor_tensor(
                            stage2_sb[:, ko:ke, :], stage2_sb[:, ko:ke, :],
                            b3repw_sb[:, ko:ke, :], ALU.add)
                    nc.sync.dma_start(
                        agin_r[:, ko:ke, :HID], stagebf_sb[:, ko:ke, :])
                nc.gpsimd.collective_compute(
                    "AllGather", ALU.bypass,
                    replica_groups=rg, ins=[agin_d[:]], outs=[agout_d[:]])

                # ---- phase 2: edge pass, slab by slab ----
                pchunk = {}
                slab_ps = [None]
                cur_slab = -1

                def flush_slab(slab):
                    if slab_ps[0] is None:
                        pchunk.clear()
                        return
                    s0 = slab * c.S
                    v2 = work.tile([128, c.S, HID], F32, tag="v2")
                    nc.vector.tensor_tensor(
                        v2[:], slab_ps[0][:], dinvrep_sb[:, s0:s0 + c.S, :],
                        ALU.mult)
                    nc.vector.tensor_tensor(
                        v2[:], v2[:], stage2_sb[:, s0:s0 + c.S, :], ALU.add)
                    if l < 2:
                        for cc in range(c.S):
                            ch = s0 + cc
                            psT = psB.tile([HID, 128], F32, space="PSUM")
                            nc.tensor.transpose(psT[:], v2[:, cc, :], ident[:])
                            nc.scalar.activation(
                                hT_sb[:, ch * 128:(ch + 1) * 128], psT[:],
                                AF.Relu, bias=bcols_sb[:, l:l + 1])
                    else:
                        nc.scalar.activation(
                            h3_sb[:, s0:s0 + c.S, :], v2[:], AF.Relu)
                    pchunk.clear()

                for call in calls:
                    s, b, n, col0 = (call["slab"], call["bucket"],
                                     call["n"], call["col"])
                    if s != cur_slab:
                        flush_slab(cur_slab)
                        slab_ps[0] = None
                        cur_slab = s
                    gi = work.tile([128, c.MAXG // 16], I16, tag="gi",
                                   name=f"gi_{l}_{col0}")
                    nc.sync.dma_start(
                        gi[:, : n // 16],
                        gidx_d[:, col0 // 16:(col0 + n) // 16])
                    msg = msgs.tile([128, c.MAXG // 128, EL], BF16, tag="msg",
                                    name=f"msg_{l}_{col0}")
                    set_nreg(n)
                    nc.gpsimd.dma_gather(
                        msg[:, : n // 128, :],
                        agout_d[b * c.SRCW:(b + 1) * c.SRCW, :],
                        gi[:, : n // 16], n, nreg, EL)
                    nr = len(call["runs"])
                    if nr:
                        rid0 = call["runs"][0][4]
                        oht = ohp.tile([128, MAXRUNS, 128], BF16, tag="oht",
                                       name=f"oht_{l}_{col0}")
                        nc.sync.dma_start(
                            oht[:, :nr, :],
                            ohruns_d[rid0 * 128:(rid0 + nr) * 128, :]
                            .rearrange("(r p) f -> p r f", p=128))
                    for w, ch, a, e, rid, first, lastf in call["runs"]:
                        if first:
                            assert ch not in pchunk
                            if slab_ps[0] is None:
                                slab_ps[0] = psC.tile(
                                    [128, c.S, HID], F32, space="PSUM",
                                    tag="pc", name=f"pc_{l}_{s}")
                            cc = ch - s * c.S
                            pchunk[ch] = slab_ps[0][:, cc, :]
                        nc.tensor.matmul(
                            pchunk[ch], oht[:, rid - rid0, :],
                            msg[:, w, :HID],
                            start=first, stop=lastf)
                flush_slab(cur_slab)

            # ---- mean pool ----
            pp = psA.tile([128, HID], F32, space="PSUM", tag="pool", bufs=1)
            for k in range(c.NCHK):
                oh = work.tile([128, 128], BF16, tag="oh2")
                nc.sync.dma_start(oh[:], oneh_d[k * 128:(k + 1) * 128, :])
                nc.tensor.matmul(
                    pp[:], oh[:], h3_sb[:, k, :],
                    start=(k == 0), stop=(k == c.NCHK - 1))
            pl = res.tile([128, HID], F32)
            nc.vector.tensor_copy(pl[:], pp[:])
            nc.sync.dma_start(plin_d[:], pl[:])
            nc.gpsimd.collective_compute(
                "AllReduce", ALU.add,
                replica_groups=rg, ins=[plin_d[:]], outs=[plout_d[:]])
            plr = res.tile([128, HID], F32)
            nc.sync.dma_start(plr[:], plout_d[:])
            plm = res.tile([128, HID], F32)
            nc.vector.tensor_scalar_mul(plm[:], plr[:], cinv_sb[:])
            psT = psB.tile([HID, 128], F32, space="PSUM", tag="pT", bufs=1)
            nc.tensor.transpose(psT[:], plm[:], ident[:])
            plT = res.tile([HID, 128], F32)
            nc.vector.tensor_copy(plT[:], psT[:])
            psD = psB.tile([G, OUT], F32, space="PSUM", tag="pC", bufs=1)
            nc.tensor.matmul(psD[:], plT[:, :G], Wc_sb[:], start=True, stop=True)
            lg = res.tile([G, OUT], F32)
            nc.vector.tensor_add(lg[:], psD[:, :], bcrep_sb[:G, :])
            mx = res.tile([G, 1], F32)
            nc.vector.tensor_reduce(mx[:], lg[:], mybir.AxisListType.X, ALU.max)
            lgs = res.tile([G, OUT], F32)
            nc.vector.tensor_scalar_sub(lgs[:], lg[:], mx[:])
            ex = res.tile([G, OUT], F32)
            nc.scalar.activation(ex[:], lgs[:], AF.Exp)
            sm = res.tile([G, 1], F32)
            nc.vector.tensor_reduce(sm[:], ex[:], mybir.AxisListType.X, ALU.add)
            ls = res.tile([G, 1], F32)
            nc.scalar.activation(ls[:], sm[:], AF.Ln)
            yt = res.tile([G, OUT], F32)
            nc.vector.tensor_scalar_sub(yt[:], lgs[:], ls[:])
            nc.sync.dma_start(y_d[:], yt[:])

    return nc


def _finalize(nc):
    nc.compile()
    fix_multiwait(nc)


def run(inputs, cfg, profile_dir=None):
    from concourse.bass_utils import run_bass_kernel_spmd

    in_maps, meta = prep(inputs, cfg)
    nc = build(cfg, meta)
    _finalize(nc)
    if profile_dir is not None:
        from trn_agent_boot.trn_boot import _ntff_profile_via_ctypes
        hook = _ntff_profile_via_ctypes("/opt/axon/libaxon_pjrt.so")
        with hook(profile_dir, [0]):
            res = run_bass_kernel_spmd(nc, in_maps, core_ids=list(range(cfg.C)))
    else:
        res = run_bass_kernel_spmd(nc, in_maps, core_ids=list(range(cfg.C)))
    return res.results[0]["y"]


# ---------------------------------------------------------------------------
N_NODES, N_EDGES, IN_DIM, HID_DIM, N_GRAPHS, OUT_DIM = 100_000, 1_600_000, 128, 64, 128, 3


def kernel(**inputs):
    import os
    cfg = Cfg(N_NODES, N_EDGES, IN_DIM, HID_DIM, N_GRAPHS, OUT_DIM)
    out = run(inputs, cfg, profile_dir=os.environ.get("GNN_PROFILE_DIR"))
    return np.asarray(out, np.float32)
